# revision 44
# baseline (speedup 1.0000x reference)
"""BiDAF attention-flow kernel for Trainium2 (Bass/Tile), SPMD over 8 cores.

Math (per batch element b, one NeuronCore each):
    cq[c,j] = sum_h e2[c,h] * wcq[h] * e1[j,h]
    s[c,j]  = sc[c] + sq[j] + cq[c,j]            (+ scalar biases, which
                                                  cancel in both softmaxes)
    a       = softmax_j(s)
    c2q     = a @ e1                              (B,C,H)
    b_att   = softmax_c(max_j s)
    q2c     = b_att @ e2                          (H,)
    out     = [e2, c2q, e2*c2q, e2*q2c] @ w_red.T + b_red

Device layout: everything lives transposed, [h on partitions, c free]:
    sT[j,c] (PSUM) -> P_T = exp(sT + sq[j])      (unnormalized; row max not
                                                  subtracted - fp32 range is
                                                  plenty for |s| <= ~12)
    L[c] = sum_j P_T  via ones-matmul            a = P_T / L
    c2qT[h,c] = e1.T @ P_T, scaled by 1/L at PSUM eviction
    max_j s   = partition_all_reduce-max of max-over-jt-tiles of P_T
                (exp is monotone), so E = M*exp(sc) with no transposes, and
    q2c = (sum_c E[c]*e2T[:,c]) / sum_c E[c]     accumulated unnormalized
                                                  while the PE works
    q2c folded into the last 6 k-tiles of w_red: wq4T[h,:] = q2c[h]*wrT[18+ht]

Phases (each fully unrolled; Tile pipelines across them):
    A: cq matmuls (jt-outer, ht-mid, ch-inner for stationary-weight reuse)
       + exp + running max + L ones-matmuls + sc rows
    B: c2q matmuls (ht-outer, jt-mid, ch-inner, stationary reuse),
       1/L eviction scaling; partition_all_reduce + E row in parallel
    C: reduction-layer pass 1 (k-tiles 0..17) + interleaved q2c mul-reduces
    D: pass 2 (k-tiles 18..23 + bias), add, store

Host does sharding/layout only: batch split, transposes, bf16 casts.
"""

import numpy as np
import ml_dtypes

B, Q, C, H, OUT = 8, 512, 2048, 768, 300
HT, JT, CT = H // 128, Q // 128, C // 128  # 6, 4, 16
NCH, CHW = 4, 512  # c chunks
CPT = 4  # c-tiles per chunk

bf16 = ml_dtypes.bfloat16

_CACHE = {}


def _build_bass():
    import concourse.tile as tile
    from concourse import mybir, bass_isa, library_config, bacc

    f32 = mybir.dt.float32
    b16 = mybir.dt.bfloat16
    AF = mybir.ActivationFunctionType
    OP = mybir.AluOpType

    nc = bacc.Bacc("TRN2", target_bir_lowering=False, debug=False)

    e1_d = nc.dram_tensor("e1", [Q, H], b16, kind="ExternalInput").ap()
    e1t_d = nc.dram_tensor("e1t", [H, Q], b16, kind="ExternalInput").ap()
    e2t_d = nc.dram_tensor("e2t", [H, C], b16, kind="ExternalInput").ap()
    wrt_d = nc.dram_tensor("wrt", [4 * H, OUT], b16, kind="ExternalInput").ap()
    wpk_d = nc.dram_tensor("wpk", [128, 3 * HT], f32, kind="ExternalInput").ap()
    bred_d = nc.dram_tensor("bred", [1, OUT], b16, kind="ExternalInput").ap()
    out_d = nc.dram_tensor("out", [C, OUT], f32, kind="ExternalOutput").ap()

    with tile.TileContext(nc) as tc:
        with (
            tc.tile_pool(name="singles", bufs=1) as singles,
            tc.tile_pool(name="m3", bufs=8) as m3p,
            tc.tile_pool(name="odma", bufs=4) as odp,
            tc.tile_pool(name="ps_mm", bufs=6, space="PSUM") as ps_mm,
            tc.tile_pool(name="ps_out", bufs=2, space="PSUM") as ps_out,
        ):
            # gpsimd: need the 'attn' ucode library for partition_all_reduce
            nc.gpsimd.load_library(library_config.attn)

            # ---- persistent SBUF tensors -------------------------------
            e1_sb = singles.tile([128, JT, H], b16)      # emb1, j on parts
            e1t_sb = singles.tile([128, HT, Q], b16)     # emb1.T, h on parts
            e1w_sb = singles.tile([128, HT, Q], b16)     # wcq * emb1.T
            e2t_sb = singles.tile([128, HT, C], b16)     # emb2.T, h on parts
            wrt_sb = singles.tile([128, 24, OUT], b16)   # w_red.T, f on parts
            wq4_sb = singles.tile([128, HT, OUT], b16)   # q2c-folded wrT tail
            wsum_sb = singles.tile([128, HT, OUT], b16)  # wrT[0:6] + wq4T
            wpk_sb = singles.tile([128, 3 * HT], f32)
            wq_sb = singles.tile([128, HT], b16)
            bred_sb = singles.tile([1, OUT], b16)
            ones_mat = singles.tile([128, 128], b16)
            ones_row_b = singles.tile([1, 128], b16)
            ones_row_f = singles.tile([1, 128], f32)
            sq_sb = singles.tile([128, JT], f32)         # sq as columns
            escb_sb = singles.tile([128, C], b16)        # exp(sc) bcast
            wc_mat = singles.tile([128, HT, 128], b16)   # wc[h] rank-1 bcast
            pt_sb = singles.tile([128, JT, NCH, CHW], b16)  # P_T = exp(sT+sq)
            c2q_sb = singles.tile([128, HT, C], b16)     # c2qT (normalized)
            macc = singles.tile([128, C], b16)           # col-max of P_T
            mall = singles.tile([128, C], b16)           # after all-reduce
            ebc_sb = singles.tile([128, C], b16)         # E bcast over parts
            s_parts = singles.tile([1, NCH], f32)
            s_sum = singles.tile([1, 1], f32)
            rs_sum = singles.tile([1, 1], f32)
            rs_col = singles.tile([128, 1], f32)
            bcr_sb = singles.tile([128, C], f32)         # 1/L bcast over parts
            u_sb = singles.tile([128, HT, NCH], f32)     # unnormalized q2c
            q2c_sb = singles.tile([128, HT], f32)
            out_sb = singles.tile([128, CT, OUT], f32)   # pass-1 partials

            # ---- loads (ordered for earliest PE start) -----------------
            nc.sync.dma_start(
                out=e1t_sb, in_=e1t_d.rearrange("(t p) j -> p t j", p=128)
            )
            nc.sync.dma_start(out=wpk_sb, in_=wpk_d)
            wcq_sb = wpk_sb[:, 0:HT]
            nc.vector.memset(ones_mat, 1.0)
            nc.vector.memset(ones_row_b, 1.0)
            nc.vector.memset(ones_row_f, 1.0)
            nc.vector.memset(macc, 0.0)
            nc.vector.tensor_copy(wq_sb, wpk_sb[:, 2 * HT : 3 * HT])
            for ht in range(HT):
                nc.vector.tensor_scalar_mul(
                    wc_mat[:, ht, :], ones_mat,
                    wpk_sb[:, HT + ht : HT + ht + 1],
                )
            # HAM warm-up: keep the PE busy while inputs stream in, so the
            # clock gate is at 8/8 when the real matmuls start
            wps = ps_mm.tile([128, CHW], f32, tag="mm", name="warm")
            for _ in range(70):
                nc.tensor.matmul(wps[:, 0:128], ones_mat, ones_mat,
                                 start=True, stop=True)
            nc.vector.tensor_copy(rs_col, wps[:, 0:1])
            e2t_r = e2t_d.rearrange("(t p) c -> p t c", p=128)
            for ht in range(HT):
                # e1w = wcq (per h) * e1T
                nc.vector.tensor_scalar_mul(
                    e1w_sb[:, ht, :], e1t_sb[:, ht, :], wcq_sb[:, ht : ht + 1]
                )
            for hf in range(2):
                fsl = slice(hf * 2 * CHW, (hf + 1) * 2 * CHW)
                for ht in range(HT):
                    nc.sync.dma_start(
                        out=e2t_sb[:, ht, fsl], in_=e2t_r[:, ht, fsl]
                    )
            nc.sync.dma_start(
                out=e1_sb, in_=e1_d.rearrange("(t p) h -> p t h", p=128)
            )
            nc.sync.dma_start(
                out=wrt_sb, in_=wrt_d.rearrange("(t p) o -> p t o", p=128)
            )
            nc.sync.dma_start(out=bred_sb, in_=bred_d)

            # ---- sq columns (tiny, feeds exp bias) ---------------------
            for jt in range(JT):
                ps = ps_mm.tile([128, CHW], f32, tag="mm")
                for ht in range(HT):
                    nc.tensor.matmul(
                        ps[:, 0:1],
                        e1t_sb[:, ht, jt * 128 : (jt + 1) * 128],
                        wq_sb[:, ht : ht + 1],
                        start=(ht == 0),
                        stop=(ht == HT - 1),
                    )
                nc.vector.tensor_copy(sq_sb[:, jt : jt + 1], ps[:, 0:1])

            # ---- phase A: sT matmuls, exp, running max -----------------
            # jt outer / ht mid / ch inner: e1w stationary tile is reused
            # across the 4 chunks (one LDWEIGHTS per (jt, ht)).
            for jt in range(JT):
                sps = [
                    ps_mm.tile([128, CHW], f32, tag="mm", name=f"sps{jt}_{i}")
                    for i in range(NCH)
                ]
                for ht in range(HT):
                    for ch in range(NCH):
                        nc.tensor.matmul(
                            sps[ch],
                            e1w_sb[:, ht, jt * 128 : (jt + 1) * 128],
                            e2t_sb[:, ht, ch * CHW : (ch + 1) * CHW],
                            start=(ht == 0),
                            stop=(ht == HT - 1),
                        )
                for ch in range(NCH):
                    csl = slice(ch * CHW, (ch + 1) * CHW)
                    nc.scalar.activation(
                        out=pt_sb[:, jt, ch, :], in_=sps[ch], func=AF.Exp,
                        bias=sq_sb[:, jt : jt + 1], scale=1.0,
                    )
                    nc.vector.tensor_max(
                        macc[:, csl], macc[:, csl], pt_sb[:, jt, ch, :]
                    )

            # ---- 1/L, already broadcast across partitions --------------
            # ones-matrix stationary: out[m,c] = sum_j P_T[j,c] for every m,
            # i.e. L[c] replicated on all 128 partitions, in one matmul per
            # (jt, chunk); then a fast reciprocal straight into bcr.
            for ch in range(NCH):
                csl = slice(ch * CHW, (ch + 1) * CHW)
                lps = ps_mm.tile([128, CHW], f32, tag="mm", name=f"lps{ch}")
                for jt in range(JT):
                    nc.tensor.matmul(
                        lps, ones_mat, pt_sb[:, jt, ch, :],
                        start=(jt == 0), stop=(jt == JT - 1),
                    )
                nc.vector.reciprocal_approx_fast(out=bcr_sb[:, csl], in_=lps)

            # ---- exp(sc), broadcast across partitions (rank-1 weights) -
            for ch in range(NCH):
                csl = slice(ch * CHW, (ch + 1) * CHW)
                ps = ps_mm.tile([128, CHW], f32, tag="mm")
                for ht in range(HT):
                    nc.tensor.matmul(
                        ps,
                        wc_mat[:, ht, :],
                        e2t_sb[:, ht, csl],
                        start=(ht == 0),
                        stop=(ht == HT - 1),
                    )
                nc.scalar.activation(
                    out=escb_sb[:, csl], in_=ps, func=AF.Exp,
                    bias=0.0, scale=1.0,
                )

            # ---- phase B: c2qT matmuls with 1/L eviction scaling -------
            # ht outer / jt mid / ch inner: e1 stationary tile reused
            # across the 4 chunks (one LDWEIGHTS per (ht, jt)).
            for ht in range(HT):
                cps = [
                    ps_mm.tile([128, CHW], f32, tag="mm", name=f"cps{ht}_{i}")
                    for i in range(NCH)
                ]
                for jt in range(JT):
                    for ch in range(NCH):
                        nc.tensor.matmul(
                            cps[ch],
                            e1_sb[:, jt, ht * 128 : (ht + 1) * 128],
                            pt_sb[:, jt, ch, :],
                            start=(jt == 0),
                            stop=(jt == JT - 1),
                        )
                for ch in range(NCH):
                    csl = slice(ch * CHW, (ch + 1) * CHW)
                    nc.vector.tensor_mul(
                        c2q_sb[:, ht, csl], cps[ch], bcr_sb[:, csl]
                    )

            # ---- b_att numerator: all-reduce max, E row, E bcast -------
            nc.gpsimd.partition_all_reduce(
                mall, macc, channels=128, reduce_op=bass_isa.ReduceOp.max
            )
            for ch in range(NCH):
                csl = slice(ch * CHW, (ch + 1) * CHW)
                nc.vector.tensor_mul(
                    ebc_sb[:, csl], mall[:, csl], escb_sb[:, csl]
                )
                nc.vector.reduce_sum(
                    out=s_parts[:, ch : ch + 1], in_=ebc_sb[0:1, csl],
                    axis=mybir.AxisListType.X,
                )

            # ---- phase C/D: reduction pass 1, q2c work, pass 2 ---------
            # pass-2 for chunk ch-1 is emitted after pass-1 of chunk ch so
            # the PE never waits on wq4 (ready while chunk 0/1 pass-1 runs),
            # and output DMAs spread across the tail.
            def pass2(ch):
                for lc in range(CPT):
                    ct = ch * CPT + lc
                    tsl = slice(ct * 128, (ct + 1) * 128)
                    obs = ps_out.tile([128, OUT], f32, tag="out", name=f"obs{ct}")
                    for ht in range(HT):
                        nc.tensor.matmul(
                            obs, e2t_sb[:, ht, tsl], wq4_sb[:, ht, :],
                            start=(ht == 0), stop=False,
                        )
                    nc.tensor.matmul(
                        obs, ones_row_b, bred_sb, start=False, stop=True,
                    )
                    od = odp.tile([128, OUT], f32, tag="od", name=f"od{ct}")
                    nc.vector.tensor_add(od, obs, out_sb[:, ct, :])
                    nc.sync.dma_start(out=out_d[tsl, :], in_=od)

            def pass1(ch):
                csl = slice(ch * CHW, (ch + 1) * CHW)
                m3s = []
                for ht in range(HT):
                    m3 = m3p.tile([128, CHW], b16, tag="m3", name=f"m3_{ch}_{ht}")
                    nc.vector.tensor_mul(
                        m3, e2t_sb[:, ht, csl], c2q_sb[:, ht, csl]
                    )
                    m3s.append(m3)
                for lc in range(CPT):
                    ct = ch * CPT + lc
                    tsl = slice(ct * 128, (ct + 1) * 128)
                    lsl = slice(ch * CHW + lc * 128, ch * CHW + (lc + 1) * 128)
                    ops = ps_out.tile([128, OUT], f32, tag="out", name=f"ops{ct}")
                    for ht in range(HT):
                        nc.tensor.matmul(
                            ops, e2t_sb[:, ht, tsl], wrt_sb[:, ht, :],
                            start=(ht == 0), stop=False,
                        )
                    for ht in range(HT):
                        nc.tensor.matmul(
                            ops, c2q_sb[:, ht, lsl], wrt_sb[:, 6 + ht, :],
                            start=False, stop=False,
                        )
                    for ht in range(HT):
                        nc.tensor.matmul(
                            ops, m3s[ht][:, lc * 128 : (lc + 1) * 128],
                            wrt_sb[:, 12 + ht, :],
                            start=False, stop=(ht == HT - 1),
                        )
                    nc.scalar.copy(out_sb[:, ct, :], ops)
                    emit_amr(3)

            amr_jobs = [
                (ht, ch2) for ch2 in range(NCH) for ht in range(HT)
            ]

            def emit_amr(n):
                for _ in range(n):
                    if not amr_jobs:
                        return
                    ht, ch2 = amr_jobs.pop(0)
                    csl2 = slice(ch2 * CHW, (ch2 + 1) * CHW)
                    m3 = m3p.tile(
                        [128, CHW], b16, tag="m3", name=f"am{ch2}_{ht}"
                    )
                    nc.vector.affine_mul_reduce(
                        out=m3,
                        accum_out=u_sb[:, ht, ch2 : ch2 + 1],
                        in0=e2t_sb[:, ht, csl2],
                        in1=ebc_sb[:, csl2],
                        scale=1.0,
                        bias=0.0,
                    )

            pass1(0)
            pass1(1)
            emit_amr(24)
            # q2c finalize: q2c = U / S, fold into wrT tail
            nc.vector.reduce_sum(
                out=s_sum, in_=s_parts, axis=mybir.AxisListType.X
            )
            nc.vector.reciprocal_approx_fast(out=rs_sum, in_=s_sum)
            rps = ps_out.tile([128, CHW], f32, tag="out")
            nc.tensor.matmul(
                rps[:, 0:1], ones_row_f, rs_sum, start=True, stop=True
            )
            nc.vector.tensor_copy(rs_col, rps[:, 0:1])
            nc.vector.reduce_sum(
                out=q2c_sb, in_=u_sb, axis=mybir.AxisListType.X
            )
            nc.vector.tensor_scalar_mul(q2c_sb, q2c_sb, rs_col)
            for ht in range(HT):
                nc.vector.tensor_scalar_mul(
                    wq4_sb[:, ht, :], wrt_sb[:, 18 + ht, :],
                    q2c_sb[:, ht : ht + 1],
                )
                nc.vector.tensor_add(
                    wsum_sb[:, ht, :], wq4_sb[:, ht, :], wrt_sb[:, ht, :]
                )

            def fused_pass(ch):
                csl = slice(ch * CHW, (ch + 1) * CHW)
                m3s = []
                for ht in range(HT):
                    m3 = m3p.tile([128, CHW], b16, tag="m3", name=f"m3f{ch}_{ht}")
                    nc.vector.tensor_mul(
                        m3, e2t_sb[:, ht, csl], c2q_sb[:, ht, csl]
                    )
                    m3s.append(m3)
                for lc in range(CPT):
                    ct = ch * CPT + lc
                    tsl = slice(ct * 128, (ct + 1) * 128)
                    ops = ps_out.tile([128, OUT], f32, tag="out", name=f"opf{ct}")
                    for ht in range(HT):
                        nc.tensor.matmul(
                            ops, e2t_sb[:, ht, tsl], wsum_sb[:, ht, :],
                            start=(ht == 0), stop=False,
                        )
                    for ht in range(HT):
                        nc.tensor.matmul(
                            ops, c2q_sb[:, ht, ch * CHW + lc * 128 :
                                        ch * CHW + (lc + 1) * 128],
                            wrt_sb[:, 6 + ht, :], start=False, stop=False,
                        )
                    for ht in range(HT):
                        nc.tensor.matmul(
                            ops, m3s[ht][:, lc * 128 : (lc + 1) * 128],
                            wrt_sb[:, 12 + ht, :], start=False, stop=False,
                        )
                    nc.tensor.matmul(
                        ops, ones_row_b, bred_sb, start=False, stop=True,
                    )
                    od = odp.tile([128, OUT], f32, tag="od", name=f"odf{ct}")
                    nc.vector.tensor_copy(od, ops)
                    nc.sync.dma_start(out=out_d[tsl, :], in_=od)

            fused_pass(2)
            pass2(0)
            pass2(1)
            fused_pass(3)

    nc.compile()
    return nc


def _get_nc():
    if "nc" not in _CACHE:
        _CACHE["nc"] = _build_bass()
    return _CACHE["nc"]


def _in_maps(emb1, emb2, w_c, b_c, w_q, b_q, w_cq, b_cq, w_red, b_red):
    # host-side sharding + layout only: batch split, transposes, bf16 casts
    emb1 = np.asarray(emb1, np.float32)
    emb2 = np.asarray(emb2, np.float32)
    wcq = np.asarray(w_cq, np.float32).reshape(HT, 128).T
    wc = np.asarray(w_c, np.float32).reshape(HT, 128).T
    wq = np.asarray(w_q, np.float32).reshape(HT, 128).T
    wpk = np.ascontiguousarray(np.concatenate([wcq, wc, wq], axis=1))
    wrt = np.ascontiguousarray(np.asarray(w_red, np.float32).T).astype(bf16)
    bred = np.asarray(b_red, np.float32).reshape(1, OUT).astype(bf16)
    maps = []
    for b in range(B):
        maps.append(
            {
                "e1": emb1[b].astype(bf16),
                "e1t": np.ascontiguousarray(emb1[b].T).astype(bf16),
                "e2t": np.ascontiguousarray(emb2[b].T).astype(bf16),
                "wrt": wrt,
                "wpk": wpk,
                "bred": bred,
            }
        )
    return maps


def run(inputs, trace=False):
    from concourse.bass_utils import run_bass_kernel_spmd

    nc = _get_nc()
    maps = _in_maps(**inputs)
    res = run_bass_kernel_spmd(nc, maps, list(range(B)), trace=trace)
    out = np.stack([res.results[b]["out"] for b in range(B)], axis=0)
    return out.astype(np.float32), res


def kernel(**inputs) -> np.ndarray:
    out, _ = run(inputs, trace=False)
    return out


# revision 45
# speedup vs baseline: 1.0046x; 1.0046x over previous
"""BiDAF attention-flow kernel for Trainium2 (Bass/Tile), SPMD over 8 cores.

Math (per batch element b, one NeuronCore each):
    cq[c,j] = sum_h e2[c,h] * wcq[h] * e1[j,h]
    s[c,j]  = sc[c] + sq[j] + cq[c,j]            (+ scalar biases, which
                                                  cancel in both softmaxes)
    a       = softmax_j(s)
    c2q     = a @ e1                              (B,C,H)
    b_att   = softmax_c(max_j s)
    q2c     = b_att @ e2                          (H,)
    out     = [e2, c2q, e2*c2q, e2*q2c] @ w_red.T + b_red

Device layout: everything lives transposed, [h on partitions, c free]:
    sT[j,c] (PSUM) -> P_T = exp(sT + sq[j])      (unnormalized; row max not
                                                  subtracted - fp32 range is
                                                  plenty for |s| <= ~12)
    L[c] = sum_j P_T  via ones-matmul            a = P_T / L
    c2qT[h,c] = e1.T @ P_T, scaled by 1/L at PSUM eviction
    max_j s   = partition_all_reduce-max of max-over-jt-tiles of P_T
                (exp is monotone), so E = M*exp(sc) with no transposes, and
    q2c = (sum_c E[c]*e2T[:,c]) / sum_c E[c]     accumulated unnormalized
                                                  while the PE works
    q2c folded into the last 6 k-tiles of w_red: wq4T[h,:] = q2c[h]*wrT[18+ht]

Phases (each fully unrolled; Tile pipelines across them):
    A: cq matmuls (jt-outer, ht-mid, ch-inner for stationary-weight reuse)
       + exp + running max + L ones-matmuls + sc rows
    B: c2q matmuls (ht-outer, jt-mid, ch-inner, stationary reuse),
       1/L eviction scaling; partition_all_reduce + E row in parallel
    C: reduction-layer pass 1 (k-tiles 0..17) + interleaved q2c mul-reduces
    D: pass 2 (k-tiles 18..23 + bias), add, store

Host does sharding/layout only: batch split, transposes, bf16 casts.
"""

import numpy as np
import ml_dtypes

B, Q, C, H, OUT = 8, 512, 2048, 768, 300
HT, JT, CT = H // 128, Q // 128, C // 128  # 6, 4, 16
NCH, CHW = 4, 512  # c chunks
CPT = 4  # c-tiles per chunk

bf16 = ml_dtypes.bfloat16

_CACHE = {}


def _build_bass():
    import concourse.tile as tile
    from concourse import mybir, bass_isa, library_config, bacc

    f32 = mybir.dt.float32
    b16 = mybir.dt.bfloat16
    AF = mybir.ActivationFunctionType
    OP = mybir.AluOpType

    nc = bacc.Bacc("TRN2", target_bir_lowering=False, debug=False)

    e1_d = nc.dram_tensor("e1", [Q, H], b16, kind="ExternalInput").ap()
    e1t_d = nc.dram_tensor("e1t", [H, Q], b16, kind="ExternalInput").ap()
    e2t_d = nc.dram_tensor("e2t", [H, C], b16, kind="ExternalInput").ap()
    wrt_d = nc.dram_tensor("wrt", [4 * H, OUT], b16, kind="ExternalInput").ap()
    wpk_d = nc.dram_tensor("wpk", [128, 3 * HT], f32, kind="ExternalInput").ap()
    bred_d = nc.dram_tensor("bred", [1, OUT], b16, kind="ExternalInput").ap()
    out_d = nc.dram_tensor("out", [C, OUT], f32, kind="ExternalOutput").ap()

    with tile.TileContext(nc) as tc:
        with (
            tc.tile_pool(name="singles", bufs=1) as singles,
            tc.tile_pool(name="m3", bufs=8) as m3p,
            tc.tile_pool(name="odma", bufs=4) as odp,
            tc.tile_pool(name="ps_mm", bufs=6, space="PSUM") as ps_mm,
            tc.tile_pool(name="ps_out", bufs=2, space="PSUM") as ps_out,
        ):
            # gpsimd: need the 'attn' ucode library for partition_all_reduce
            nc.gpsimd.load_library(library_config.attn)

            # ---- persistent SBUF tensors -------------------------------
            e1_sb = singles.tile([128, JT, H], b16)      # emb1, j on parts
            e1t_sb = singles.tile([128, HT, Q], b16)     # emb1.T, h on parts
            e1w_sb = singles.tile([128, HT, Q], b16)     # wcq * emb1.T
            e2t_sb = singles.tile([128, HT, C], b16)     # emb2.T, h on parts
            wrt_sb = singles.tile([128, 24, OUT], b16)   # w_red.T, f on parts
            wq4_sb = singles.tile([128, HT, OUT], b16)   # q2c-folded wrT tail
            wsum_sb = singles.tile([128, HT, OUT], b16)  # wrT[0:6] + wq4T
            wpk_sb = singles.tile([128, 3 * HT], f32)
            wq_sb = singles.tile([128, HT], b16)
            bred_sb = singles.tile([1, OUT], b16)
            ones_mat = singles.tile([128, 128], b16)
            ones_row_b = singles.tile([1, 128], b16)
            ones_row_f = singles.tile([1, 128], f32)
            sq_sb = singles.tile([128, JT], f32)         # sq as columns
            escb_sb = singles.tile([128, C], b16)        # exp(sc) bcast
            wc_mat = singles.tile([128, HT, 128], b16)   # wc[h] rank-1 bcast
            pt_sb = singles.tile([128, JT, NCH, CHW], b16)  # P_T = exp(sT+sq)
            c2q_sb = singles.tile([128, HT, C], b16)     # c2qT (normalized)
            macc = singles.tile([128, C], b16)           # col-max of P_T
            mall = singles.tile([128, C], b16)           # after all-reduce
            ebc_sb = singles.tile([128, C], b16)         # E bcast over parts
            s_parts = singles.tile([1, NCH], f32)
            s_sum = singles.tile([1, 1], f32)
            rs_sum = singles.tile([1, 1], f32)
            rs_col = singles.tile([128, 1], f32)
            bcr_sb = singles.tile([128, C], f32)         # 1/L bcast over parts
            u_sb = singles.tile([128, HT, NCH], f32)     # unnormalized q2c
            q2c_sb = singles.tile([128, HT], f32)
            out_sb = singles.tile([128, CT, OUT], f32)   # pass-1 partials

            # ---- loads (ordered for earliest PE start) -----------------
            nc.sync.dma_start(
                out=e1t_sb, in_=e1t_d.rearrange("(t p) j -> p t j", p=128)
            )
            nc.sync.dma_start(out=wpk_sb, in_=wpk_d)
            wcq_sb = wpk_sb[:, 0:HT]
            nc.vector.memset(ones_mat, 1.0)
            nc.vector.memset(ones_row_b, 1.0)
            nc.vector.memset(ones_row_f, 1.0)
            nc.vector.memset(macc, 0.0)
            nc.vector.tensor_copy(wq_sb, wpk_sb[:, 2 * HT : 3 * HT])
            for ht in range(HT):
                nc.vector.tensor_scalar_mul(
                    wc_mat[:, ht, :], ones_mat,
                    wpk_sb[:, HT + ht : HT + ht + 1],
                )
            # HAM warm-up: keep the PE busy while inputs stream in, so the
            # clock gate is at 8/8 when the real matmuls start
            wps = ps_mm.tile([128, CHW], f32, tag="mm", name="warm")
            for _ in range(70):
                nc.tensor.matmul(wps[:, 0:128], ones_mat, ones_mat,
                                 start=True, stop=True)
            nc.vector.tensor_copy(rs_col, wps[:, 0:1])
            e2t_r = e2t_d.rearrange("(t p) c -> p t c", p=128)
            for ht in range(HT):
                # e1w = wcq (per h) * e1T
                nc.vector.tensor_scalar_mul(
                    e1w_sb[:, ht, :], e1t_sb[:, ht, :], wcq_sb[:, ht : ht + 1]
                )
            for hf in range(2):
                fsl = slice(hf * 2 * CHW, (hf + 1) * 2 * CHW)
                for ht in range(HT):
                    nc.sync.dma_start(
                        out=e2t_sb[:, ht, fsl], in_=e2t_r[:, ht, fsl]
                    )
            nc.sync.dma_start(
                out=e1_sb, in_=e1_d.rearrange("(t p) h -> p t h", p=128)
            )
            nc.sync.dma_start(
                out=wrt_sb, in_=wrt_d.rearrange("(t p) o -> p t o", p=128)
            )
            nc.sync.dma_start(out=bred_sb, in_=bred_d)

            # ---- sq columns (tiny, feeds exp bias) ---------------------
            for jt in range(JT):
                ps = ps_mm.tile([128, CHW], f32, tag="mm")
                for ht in range(HT):
                    nc.tensor.matmul(
                        ps[:, 0:1],
                        e1t_sb[:, ht, jt * 128 : (jt + 1) * 128],
                        wq_sb[:, ht : ht + 1],
                        start=(ht == 0),
                        stop=(ht == HT - 1),
                    )
                nc.vector.tensor_copy(sq_sb[:, jt : jt + 1], ps[:, 0:1])

            # ---- phase A: sT matmuls, exp, running max -----------------
            # jt outer / ht mid / ch inner: e1w stationary tile is reused
            # across the 4 chunks (one LDWEIGHTS per (jt, ht)).
            for jt in range(JT):
                sps = [
                    ps_mm.tile([128, CHW], f32, tag="mm", name=f"sps{jt}_{i}")
                    for i in range(NCH)
                ]
                for ht in range(HT):
                    for ch in range(NCH):
                        nc.tensor.matmul(
                            sps[ch],
                            e1w_sb[:, ht, jt * 128 : (jt + 1) * 128],
                            e2t_sb[:, ht, ch * CHW : (ch + 1) * CHW],
                            start=(ht == 0),
                            stop=(ht == HT - 1),
                        )
                for ch in range(NCH):
                    csl = slice(ch * CHW, (ch + 1) * CHW)
                    nc.scalar.activation(
                        out=pt_sb[:, jt, ch, :], in_=sps[ch], func=AF.Exp,
                        bias=sq_sb[:, jt : jt + 1], scale=1.0,
                    )
                    nc.vector.tensor_max(
                        macc[:, csl], macc[:, csl], pt_sb[:, jt, ch, :]
                    )

            # ---- 1/L, already broadcast across partitions --------------
            # ones-matrix stationary: out[m,c] = sum_j P_T[j,c] for every m,
            # i.e. L[c] replicated on all 128 partitions, in one matmul per
            # (jt, chunk); then a fast reciprocal straight into bcr.
            for ch in range(NCH):
                csl = slice(ch * CHW, (ch + 1) * CHW)
                lps = ps_mm.tile([128, CHW], f32, tag="mm", name=f"lps{ch}")
                for jt in range(JT):
                    nc.tensor.matmul(
                        lps, ones_mat, pt_sb[:, jt, ch, :],
                        start=(jt == 0), stop=(jt == JT - 1),
                    )
                nc.vector.reciprocal_approx_fast(out=bcr_sb[:, csl], in_=lps)

            # ---- exp(sc), broadcast across partitions (rank-1 weights) -
            for ch in range(NCH):
                csl = slice(ch * CHW, (ch + 1) * CHW)
                ps = ps_mm.tile([128, CHW], f32, tag="mm")
                for ht in range(HT):
                    nc.tensor.matmul(
                        ps,
                        wc_mat[:, ht, :],
                        e2t_sb[:, ht, csl],
                        start=(ht == 0),
                        stop=(ht == HT - 1),
                    )
                nc.scalar.activation(
                    out=escb_sb[:, csl], in_=ps, func=AF.Exp,
                    bias=0.0, scale=1.0,
                )

            # ---- phase B: c2qT matmuls with 1/L eviction scaling -------
            # ht outer / jt mid / ch inner: e1 stationary tile reused
            # across the 4 chunks (one LDWEIGHTS per (ht, jt)).
            for ht in range(HT):
                cps = [
                    ps_mm.tile([128, CHW], f32, tag="mm", name=f"cps{ht}_{i}")
                    for i in range(NCH)
                ]
                for jt in range(JT):
                    for ch in range(NCH):
                        nc.tensor.matmul(
                            cps[ch],
                            e1_sb[:, jt, ht * 128 : (ht + 1) * 128],
                            pt_sb[:, jt, ch, :],
                            start=(jt == 0),
                            stop=(jt == JT - 1),
                        )
                for ch in range(NCH):
                    csl = slice(ch * CHW, (ch + 1) * CHW)
                    nc.vector.tensor_mul(
                        c2q_sb[:, ht, csl], cps[ch], bcr_sb[:, csl]
                    )

            # ---- b_att numerator: all-reduce max, E row, E bcast -------
            nc.gpsimd.partition_all_reduce(
                mall, macc, channels=128, reduce_op=bass_isa.ReduceOp.max
            )
            for ch in range(NCH):
                csl = slice(ch * CHW, (ch + 1) * CHW)
                nc.vector.tensor_mul(
                    ebc_sb[:, csl], mall[:, csl], escb_sb[:, csl]
                )
                nc.vector.reduce_sum(
                    out=s_parts[:, ch : ch + 1], in_=ebc_sb[0:1, csl],
                    axis=mybir.AxisListType.X,
                )
            nc.vector.reduce_sum(
                out=s_sum, in_=s_parts, axis=mybir.AxisListType.X
            )
            nc.vector.reciprocal_approx_fast(out=rs_sum, in_=s_sum)
            rps = ps_out.tile([128, CHW], f32, tag="out")
            nc.tensor.matmul(
                rps[:, 0:1], ones_row_f, rs_sum, start=True, stop=True
            )
            nc.vector.tensor_copy(rs_col, rps[:, 0:1])

            # ---- phase C/D: reduction pass 1, q2c work, pass 2 ---------
            # pass-2 for chunk ch-1 is emitted after pass-1 of chunk ch so
            # the PE never waits on wq4 (ready while chunk 0/1 pass-1 runs),
            # and output DMAs spread across the tail.
            def pass2(ch):
                for lc in range(CPT):
                    ct = ch * CPT + lc
                    tsl = slice(ct * 128, (ct + 1) * 128)
                    obs = ps_out.tile([128, OUT], f32, tag="out", name=f"obs{ct}")
                    for ht in range(HT):
                        nc.tensor.matmul(
                            obs, e2t_sb[:, ht, tsl], wq4_sb[:, ht, :],
                            start=(ht == 0), stop=False,
                        )
                    nc.tensor.matmul(
                        obs, ones_row_b, bred_sb, start=False, stop=True,
                    )
                    od = odp.tile([128, OUT], f32, tag="od", name=f"od{ct}")
                    nc.vector.tensor_add(od, obs, out_sb[:, ct, :])
                    nc.sync.dma_start(out=out_d[tsl, :], in_=od)

            def pass1(ch):
                csl = slice(ch * CHW, (ch + 1) * CHW)
                m3s = []
                for ht in range(HT):
                    m3 = m3p.tile([128, CHW], b16, tag="m3", name=f"m3_{ch}_{ht}")
                    nc.vector.tensor_mul(
                        m3, e2t_sb[:, ht, csl], c2q_sb[:, ht, csl]
                    )
                    m3s.append(m3)
                for lc in range(CPT):
                    ct = ch * CPT + lc
                    tsl = slice(ct * 128, (ct + 1) * 128)
                    lsl = slice(ch * CHW + lc * 128, ch * CHW + (lc + 1) * 128)
                    ops = ps_out.tile([128, OUT], f32, tag="out", name=f"ops{ct}")
                    for ht in range(HT):
                        nc.tensor.matmul(
                            ops, e2t_sb[:, ht, tsl], wrt_sb[:, ht, :],
                            start=(ht == 0), stop=False,
                        )
                    for ht in range(HT):
                        nc.tensor.matmul(
                            ops, c2q_sb[:, ht, lsl], wrt_sb[:, 6 + ht, :],
                            start=False, stop=False,
                        )
                    for ht in range(HT):
                        nc.tensor.matmul(
                            ops, m3s[ht][:, lc * 128 : (lc + 1) * 128],
                            wrt_sb[:, 12 + ht, :],
                            start=False, stop=(ht == HT - 1),
                        )
                    nc.scalar.copy(out_sb[:, ct, :], ops)
                    emit_amr(3)

            amr_jobs = [
                (ht, ch2) for ch2 in range(NCH) for ht in range(HT)
            ]

            def emit_amr(n):
                for _ in range(n):
                    if not amr_jobs:
                        return
                    ht, ch2 = amr_jobs.pop(0)
                    csl2 = slice(ch2 * CHW, (ch2 + 1) * CHW)
                    m3 = m3p.tile(
                        [128, CHW], b16, tag="m3", name=f"am{ch2}_{ht}"
                    )
                    nc.vector.affine_mul_reduce(
                        out=m3,
                        accum_out=u_sb[:, ht, ch2 : ch2 + 1],
                        in0=e2t_sb[:, ht, csl2],
                        in1=ebc_sb[:, csl2],
                        scale=1.0,
                        bias=0.0,
                    )

            pass1(0)
            pass1(1)
            emit_amr(24)
            # q2c finalize: q2c = U / S, fold into wrT tail
            nc.vector.reduce_sum(
                out=q2c_sb, in_=u_sb, axis=mybir.AxisListType.X
            )
            nc.vector.tensor_scalar_mul(q2c_sb, q2c_sb, rs_col)
            for ht in range(HT):
                nc.vector.tensor_scalar_mul(
                    wq4_sb[:, ht, :], wrt_sb[:, 18 + ht, :],
                    q2c_sb[:, ht : ht + 1],
                )
                nc.vector.tensor_add(
                    wsum_sb[:, ht, :], wq4_sb[:, ht, :], wrt_sb[:, ht, :]
                )

            def fused_pass(ch):
                csl = slice(ch * CHW, (ch + 1) * CHW)
                m3s = []
                for ht in range(HT):
                    m3 = m3p.tile([128, CHW], b16, tag="m3", name=f"m3f{ch}_{ht}")
                    nc.vector.tensor_mul(
                        m3, e2t_sb[:, ht, csl], c2q_sb[:, ht, csl]
                    )
                    m3s.append(m3)
                for lc in range(CPT):
                    ct = ch * CPT + lc
                    tsl = slice(ct * 128, (ct + 1) * 128)
                    ops = ps_out.tile([128, OUT], f32, tag="out", name=f"opf{ct}")
                    for ht in range(HT):
                        nc.tensor.matmul(
                            ops, e2t_sb[:, ht, tsl], wsum_sb[:, ht, :],
                            start=(ht == 0), stop=False,
                        )
                    for ht in range(HT):
                        nc.tensor.matmul(
                            ops, c2q_sb[:, ht, ch * CHW + lc * 128 :
                                        ch * CHW + (lc + 1) * 128],
                            wrt_sb[:, 6 + ht, :], start=False, stop=False,
                        )
                    for ht in range(HT):
                        nc.tensor.matmul(
                            ops, m3s[ht][:, lc * 128 : (lc + 1) * 128],
                            wrt_sb[:, 12 + ht, :], start=False, stop=False,
                        )
                    nc.tensor.matmul(
                        ops, ones_row_b, bred_sb, start=False, stop=True,
                    )
                    od = odp.tile([128, OUT], f32, tag="od", name=f"odf{ct}")
                    nc.scalar.copy(od, ops)
                    nc.sync.dma_start(out=out_d[tsl, :], in_=od)

            fused_pass(2)
            pass2(0)
            pass2(1)
            fused_pass(3)

    nc.compile()
    return nc


def _get_nc():
    if "nc" not in _CACHE:
        _CACHE["nc"] = _build_bass()
    return _CACHE["nc"]


def _in_maps(emb1, emb2, w_c, b_c, w_q, b_q, w_cq, b_cq, w_red, b_red):
    # host-side sharding + layout only: batch split, transposes, bf16 casts
    emb1 = np.asarray(emb1, np.float32)
    emb2 = np.asarray(emb2, np.float32)
    wcq = np.asarray(w_cq, np.float32).reshape(HT, 128).T
    wc = np.asarray(w_c, np.float32).reshape(HT, 128).T
    wq = np.asarray(w_q, np.float32).reshape(HT, 128).T
    wpk = np.ascontiguousarray(np.concatenate([wcq, wc, wq], axis=1))
    wrt = np.ascontiguousarray(np.asarray(w_red, np.float32).T).astype(bf16)
    bred = np.asarray(b_red, np.float32).reshape(1, OUT).astype(bf16)
    maps = []
    for b in range(B):
        maps.append(
            {
                "e1": emb1[b].astype(bf16),
                "e1t": np.ascontiguousarray(emb1[b].T).astype(bf16),
                "e2t": np.ascontiguousarray(emb2[b].T).astype(bf16),
                "wrt": wrt,
                "wpk": wpk,
                "bred": bred,
            }
        )
    return maps


def run(inputs, trace=False):
    from concourse.bass_utils import run_bass_kernel_spmd

    nc = _get_nc()
    maps = _in_maps(**inputs)
    res = run_bass_kernel_spmd(nc, maps, list(range(B)), trace=trace)
    out = np.stack([res.results[b]["out"] for b in range(B)], axis=0)
    return out.astype(np.float32), res


def kernel(**inputs) -> np.ndarray:
    out, _ = run(inputs, trace=False)
    return out
